# revision 10
# baseline (speedup 1.0000x reference)
"""Single-head causal attention with per-batch padding, on 8 trn2 NeuronCores.

Problem: batch [8, 2048, 512] f32; q/k/v = x @ W.T + b; scores = q k^T / sqrt(512)
masked causal & col<length; softmax; out = attn @ v.

Sharding: data-parallel over batch — core b handles batch element b.

v3 design (over the f16 baseline):
  - Host folds A = wq.T @ wk / sqrt(D), so scores = (x A) x^T: the K projection
    disappears — the score matmul's K-side operand is raw x^T, already resident
    as the packed input. bq's column effect enters as a host-computed per-column
    bias added to the pad bias; bk's row effect is constant per softmax row and
    cancels in softmax.
  - Software-pipelined attention loop: ST for k-block k+1 is emitted BEFORE
    sums/AV of block k, so the in-order PE never head-of-line blocks on the
    scalar engine's exp. (This was ~25us of PE idle in the baseline.)
  - Output stored f16 (host upcasts); v-bias added on host (softmax rows sum
    to 1, so attn @ (V + bv) = attn @ V + bv exactly).
  - All matmuls f16 (fp8 on the score/V paths measurably breaks the 2e-2
    accuracy gate; fp8's 2x only survives on attn@V via a double-fp8 V split,
    kept behind AV_P8 for experimentation).
"""

import numpy as np

import concourse.bacc as bacc
import concourse.mybir as mybir
from concourse.tile import TileContext
from concourse.bass_utils import run_bass_kernel_spmd

B, S, D = 8, 2048, 512
P = 128
NB = S // P          # 16 row/col blocks of 128
CHUNK = 512
NCH = S // CHUNK     # 4 query chunks
KD = D // P          # 4 contraction blocks over d
N_CORES = 8
NEG = -30000.0
F32 = mybir.dt.float32
F16 = mybir.dt.float16
MMDT = F16

_cache = {}


def _build():
    nc = bacc.Bacc()
    # xp[p, c*2048 + k*512 + j] = x[c*512 + j, k*128 + p]
    xp = nc.declare_dram_parameter("xp", [P, KD * S], MMDT, isOutput=False)
    # ap[p, k*512 + j] = A_eff[k*128 + p, j], A_eff = wq.T @ wk / sqrt(D)
    ap = nc.declare_dram_parameter("ap", [P, KD * D], MMDT, isOutput=False)
    wvp = nc.declare_dram_parameter("wvp", [P, KD * D], MMDT, isOutput=False)
    # csts: cols [0:16] pad bias blocks (pad + colbias from bq, per-core)
    csts = nc.declare_dram_parameter("csts", [P, NB], F32, isOutput=False)
    tri01 = nc.declare_dram_parameter("tri01", [P, P], MMDT, isOutput=False)
    out = nc.declare_dram_parameter("out", [S, D], F16, isOutput=True)

    with TileContext(nc) as tc:
        with (
            tc.tile_pool(name="const", bufs=1) as constp,
            tc.tile_pool(name="proj", bufs=1) as projp,
            tc.tile_pool(name="st_psum", bufs=3, space="PSUM") as stp,
            tc.tile_pool(name="av_psum", bufs=1, space="PSUM") as avp,
            tc.tile_pool(name="sum_psum", bufs=1, space="PSUM") as sump,
            tc.tile_pool(name="pt", bufs=4) as ptp,
            tc.tile_pool(name="oev", bufs=3) as oevp,
            tc.tile_pool(name="sumt", bufs=2) as sumtp,
        ):
            cst = constp.tile([P, NB], F32, tag="cst")
            ones_t = constp.tile([P, 1], MMDT, tag="ones")
            nc.gpsimd.memset(ones_t[:], 1.0)
            tri_t = constp.tile([P, P], MMDT, tag="tri01")

            # x^T stays resident: it is both the projection input and the
            # score matmul's K-side operand (A-fold).
            xt_t = projp.tile([P, KD * S], MMDT, tag="xt", name="xt")
            qt_sb = [projp.tile([P, S], MMDT, tag=f"qt{m}", name=f"qt{m}")
                     for m in range(KD)]
            v_sb = [projp.tile([P, D], MMDT, tag=f"v{i}", name=f"v{i}")
                    for i in range(NB)]

            def xs(c, k):  # x^T tile [128, 512]: d-block k, s-chunk c
                o = c * 2048 + k * CHUNK
                return xt_t[:, o:o + CHUNK]

            def xk(kb, kk):  # x^T tile [128, 128]: d-block kk, s-block kb
                o = (kb // 4) * 2048 + kk * CHUNK + (kb % 4) * P
                return xt_t[:, o:o + P]

            # ---- Phase A+B: loads + projections ----
            with tc.tile_pool(name="xw", bufs=1) as xwp:
                a_t = xwp.tile([P, KD * D], MMDT, tag="a", name="a")
                wv_t = xwp.tile([P, KD * D], MMDT, tag="wv", name="wv")
                # chunk-0 x and the A pack land first, spread across four
                # engines' DMA queues so the first projection starts ASAP
                for k in range(KD):
                    eng = (nc.sync, nc.sync, nc.gpsimd, nc.gpsimd)[k]
                    eng.dma_start(
                        out=xt_t[:, k * CHUNK:(k + 1) * CHUNK],
                        in_=xp[:, k * CHUNK:(k + 1) * CHUNK])
                    nc.scalar.dma_start(
                        out=a_t[:, k * D:(k + 1) * D],
                        in_=ap[:, k * D:(k + 1) * D])
                nc.sync.dma_start(out=cst[:], in_=csts[:])
                nc.scalar.dma_start(out=wv_t[:], in_=wvp[:])
                nc.scalar.dma_start(out=tri_t[:], in_=tri01[:])
                for c in range(1, NCH):
                    for k in range(KD):
                        o = c * 2048 + k * CHUNK
                        nc.sync.dma_start(
                            out=xt_t[:, o:o + CHUNK], in_=xp[:, o:o + CHUNK])

                # Q~^T: [d_out block m, s chunk c] = sum_k A[k][:,m]^T x^T[k][:,c]
                for c in range(NCH):
                    for m in range(KD):
                        ps = stp.tile([P, CHUNK], F32, tag="pst")
                        for k in range(KD):
                            nc.tensor.matmul(
                                ps[:], a_t[:, k * D + m * P:k * D + (m + 1) * P],
                                xs(c, k), start=(k == 0), stop=(k == KD - 1))
                        nc.vector.tensor_copy(
                            qt_sb[m][:, c * CHUNK:(c + 1) * CHUNK], ps[:])
                    # V: [s block i, d] = sum_k x^T[k][:, i]^T wv[k]
                    for ii in range(4):
                        i = 4 * c + ii
                        ps = stp.tile([P, D], F32, tag="pst")
                        for k in range(KD):
                            nc.tensor.matmul(
                                ps[:], xt_t[:, c * 2048 + k * CHUNK + ii * P:
                                            c * 2048 + k * CHUNK + (ii + 1) * P],
                                wv_t[:, k * D:(k + 1) * D],
                                start=(k == 0), stop=(k == KD - 1))
                        nc.vector.tensor_copy(v_sb[i][:], ps[:])

            # ---- Phase C: attention per query chunk, 1-block ST lookahead ----
            def emit_norm_out(c, j, sums, sums_sb, ot):
                # sums j-slice -> per-partition [128,1] -> recip -> scale -> store
                nc.vector.tensor_copy(
                    sums_sb[0:1, j * P:(j + 1) * P],
                    sums[0:1, j * P:(j + 1) * P])
                eng = nc.sync if j % 2 == 0 else nc.scalar
                sums_tj = sumtp.tile([P, 1], F32, tag=f"sumt{j}",
                                     name=f"sumt{j}")
                eng.dma_start(out=sums_tj[:],
                              in_=sums_sb[0:1, j * P:(j + 1) * P])
                recip_j = sumtp.tile([P, 1], F32, tag=f"recip{j}",
                                     name=f"recip{j}")
                nc.vector.reciprocal(recip_j[:], sums_tj[:])
                if j % 2 == 0:
                    nc.vector.tensor_scalar_mul(ot[j][:], ot[j][:], recip_j[:])
                else:
                    nc.scalar.activation(
                        ot[j][:], ot[j][:],
                        mybir.ActivationFunctionType.Copy,
                        scale=recip_j[:])
                r0 = (4 * c + j) * P
                eng.dma_start(out=out[r0:r0 + P, :], in_=ot[j][:])

            for c in range(NCH):
                av = [avp.tile([P, D], F32, tag=f"av{j}", name=f"av{j}")
                      for j in range(4)]
                ot = [oevp.tile([P, D], F16, tag=f"ot{j}", name=f"ot{j}")
                      for j in range(4)]
                sums = sump.tile([1, CHUNK], F32, tag="sums")
                sums_sb = sumtp.tile([1, CHUNK], F32, tag="sums_sb")
                nkb = 4 * c + 4  # causal: sk blocks 0 .. 4c+3

                def emit_st(k):
                    # ST chunk [sk=128, sq<=512] = sum_d x^T[d,sk]^T Q~^T[d,sq];
                    # exp with pad bias; 0/1 tri multiply on the diagonal block
                    m = k - 4 * c
                    lo = max(m, 0) * P
                    st = stp.tile([P, CHUNK], F32, tag="pst")
                    for kk in range(KD):
                        nc.tensor.matmul(
                            st[:, lo:CHUNK], xk(k, kk),
                            qt_sb[kk][:, c * CHUNK + lo:(c + 1) * CHUNK],
                            start=(kk == 0), stop=(kk == KD - 1))
                    pt = ptp.tile([P, CHUNK], MMDT, tag="pt")
                    nc.scalar.activation(
                        pt[:, lo:CHUNK], st[:, lo:CHUNK],
                        mybir.ActivationFunctionType.Exp,
                        bias=cst[:, k:k + 1], scale=1.0)
                    if m >= 0:
                        nc.vector.tensor_mul(
                            pt[:, m * P:(m + 1) * P],
                            pt[:, m * P:(m + 1) * P], tri_t[:])
                    return pt

                pt_k = emit_st(0)
                for k in range(nkb):
                    m = k - 4 * c
                    lo = max(m, 0) * P
                    # lookahead: next block's ST goes to the PE queue before
                    # this block's sums/AV (which wait on the scalar engine's
                    # exp) — keeps the in-order PE busy during the exp.
                    pt_next = emit_st(k + 1) if k + 1 < nkb else None
                    nc.tensor.matmul(
                        sums[0:1, lo:CHUNK], ones_t[:], pt_k[:, lo:CHUNK],
                        start=(k == 0), stop=(k == nkb - 1))
                    for j in range(4):
                        if k <= 4 * c + j:
                            nc.tensor.matmul(
                                av[j][:], pt_k[:, j * P:(j + 1) * P], v_sb[k][:],
                                start=(k == 0), stop=(k == 4 * c + j))
                    if m >= 0:
                        # av[m] complete: evacuate unnormalized now to free
                        # its PSUM bank for the next chunk
                        if m % 2 == 0:
                            nc.scalar.activation(
                                ot[m][:], av[m][:],
                                mybir.ActivationFunctionType.Copy)
                        else:
                            nc.vector.tensor_copy(ot[m][:], av[m][:])
                        # sums cols [m*128:(m+1)*128] got their last
                        # contribution at this k (later blocks trim past
                        # them): normalize + store this q-subblock now so
                        # the end-of-chunk tail only holds the last one
                        emit_norm_out(c, m, sums, sums_sb, ot)
                    pt_k = pt_next
    nc.compile()
    return nc


def _get_nc():
    if "nc" not in _cache:
        _cache["nc"] = _build()
    return _cache["nc"]


def _in_maps(batch, wq, bq, wk, bk, wv, bv, lengths):
    wq64 = wq.astype(np.float64)
    wk64 = wk.astype(np.float64)
    a_eff = (wq64.T @ wk64) / np.sqrt(D)        # [D, D]
    colvec = (wk64.T @ bq.astype(np.float64)) / np.sqrt(D)

    def packw(m):
        # [p, k*512 + j] = m[k*128 + p, j]
        return np.ascontiguousarray(
            m.reshape(KD, P, D).transpose(1, 0, 2).reshape(P, KD * D)
        ).astype(np.float16)

    app = packw(a_eff)
    wvpp = packw(wv.astype(np.float64).T)
    tri01 = np.ascontiguousarray(np.where(
        np.arange(P)[:, None] <= np.arange(P)[None, :],
        np.float16(1), np.float16(0)))
    cols = np.arange(S)
    maps = []
    for b in range(N_CORES):
        # xp[p, c*2048 + k*512 + j] = x[c*512 + j, k*128 + p]
        xb = batch[b].astype(np.float16)
        xpk = np.ascontiguousarray(
            xb.reshape(NCH, CHUNK, KD, P).transpose(3, 0, 2, 1).reshape(P, KD * S))
        colbias = batch[b].astype(np.float64) @ colvec
        padb = np.where(cols < int(lengths[b]), 0.0, NEG) + colbias
        cst_b = np.ascontiguousarray(padb.reshape(NB, P).T.astype(np.float32))
        maps.append({"xp": xpk, "ap": app, "wvp": wvpp,
                     "csts": cst_b, "tri01": tri01})
    return maps


def _execute(in_maps, trace=False):
    nc = _get_nc()
    # always install: run_bass_kernel_spmd also honours a BASS_TRACE env var,
    # and would crash importing antenv.axon_hooks if unregistered
    _install_ntff_hook()
    return run_bass_kernel_spmd(nc, in_maps, list(range(N_CORES)), trace=trace)


def _install_ntff_hook():
    """The agent image's antenv lacks axon_hooks; register the NTFF profile
    hook ourselves so trace=True yields exec_time_ns."""
    import sys, types
    if "antenv.axon_hooks" in sys.modules:
        return
    try:
        import trn_agent_boot.trn_boot as tb
        hook = tb._ntff_profile_via_ctypes("/opt/axon/libaxon_pjrt.so")
    except Exception:
        return
    mod = types.ModuleType("antenv.axon_hooks")
    mod._hook = hook
    mod.get_axon_ntff_profile_hook = lambda: mod._hook
    mod.set_axon_ntff_profile_hook = lambda h: setattr(mod, "_hook", h)
    sys.modules["antenv.axon_hooks"] = mod
    try:
        import antenv
        antenv.axon_hooks = mod
    except Exception:
        pass


def kernel(batch, wq, bq, wk, bk, wv, bv, lengths):
    batch = np.asarray(batch)
    wq, bq = np.asarray(wq), np.asarray(bq)
    wk, bk = np.asarray(wk), np.asarray(bk)
    wv, bv = np.asarray(wv), np.asarray(bv)
    lengths = np.asarray(lengths)
    maps = _in_maps(batch, wq, bq, wk, bk, wv, bv, lengths)
    res = _execute(maps, trace=False)
    outs = [np.asarray(res.results[b]["out"]) for b in range(N_CORES)]
    full = np.stack(outs, axis=0).astype(np.float32)
    full += bv.astype(np.float32)[None, None, :]
    return full


# revision 11
# speedup vs baseline: 1.1932x; 1.1932x over previous
"""Single-head causal attention with per-batch padding, on 8 trn2 NeuronCores.

Problem: batch [8, 2048, 512] f32; q/k/v = x @ W.T + b; scores = q k^T / sqrt(512)
masked causal & col<length; softmax; out = attn @ v.

Sharding: data-parallel over batch — core b handles batch element b.

v3 design (over the f16 baseline):
  - Host folds A = wq.T @ wk / sqrt(D), so scores = (x A) x^T: the K projection
    disappears — the score matmul's K-side operand is raw x^T, already resident
    as the packed input. bq's column effect enters as a host-computed per-column
    bias added to the pad bias; bk's row effect is constant per softmax row and
    cancels in softmax.
  - Software-pipelined attention loop: ST for k-block k+1 is emitted BEFORE
    sums/AV of block k, so the in-order PE never head-of-line blocks on the
    scalar engine's exp. (This was ~25us of PE idle in the baseline.)
  - Output stored f16 (host upcasts); v-bias added on host (softmax rows sum
    to 1, so attn @ (V + bv) = attn @ V + bv exactly).
  - All matmuls f16 (fp8 on the score/V paths measurably breaks the 2e-2
    accuracy gate; fp8's 2x only survives on attn@V via a double-fp8 V split,
    kept behind AV_P8 for experimentation).
"""

import numpy as np

import concourse.bacc as bacc
import concourse.mybir as mybir
from concourse.tile import TileContext
from concourse.bass_utils import run_bass_kernel_spmd

B, S, D = 8, 2048, 512
P = 128
NB = S // P          # 16 row/col blocks of 128
CHUNK = 512
NCH = S // CHUNK     # 4 query chunks
KD = D // P          # 4 contraction blocks over d
N_CORES = 8
NEG = -30000.0
F32 = mybir.dt.float32
F16 = mybir.dt.float16
MMDT = F16

_cache = {}


def _build():
    nc = bacc.Bacc()
    # xp[p, c*2048 + k*512 + j] = x[c*512 + j, k*128 + p]
    xp = nc.declare_dram_parameter("xp", [P, KD * S], MMDT, isOutput=False)
    # ap[p, k*512 + j] = A_eff[k*128 + p, j], A_eff = wq.T @ wk / sqrt(D)
    ap = nc.declare_dram_parameter("ap", [P, KD * D], MMDT, isOutput=False)
    wvp = nc.declare_dram_parameter("wvp", [P, KD * D], MMDT, isOutput=False)
    # csts: cols [0:16] pad bias blocks (pad + colbias from bq, per-core)
    csts = nc.declare_dram_parameter("csts", [P, NB], F32, isOutput=False)
    tri01 = nc.declare_dram_parameter("tri01", [P, P], MMDT, isOutput=False)
    out = nc.declare_dram_parameter("out", [S, D], F16, isOutput=True)

    with TileContext(nc) as tc:
        with (
            tc.tile_pool(name="const", bufs=1) as constp,
            tc.tile_pool(name="proj", bufs=1) as projp,
            tc.tile_pool(name="st_psum", bufs=3, space="PSUM") as stp,
            tc.tile_pool(name="av_psum", bufs=1, space="PSUM") as avp,
            tc.tile_pool(name="sum_psum", bufs=1, space="PSUM") as sump,
            tc.tile_pool(name="pt", bufs=4) as ptp,
            tc.tile_pool(name="oev", bufs=3) as oevp,
            tc.tile_pool(name="sumt", bufs=2) as sumtp,
        ):
            cst = constp.tile([P, NB], F32, tag="cst")
            ones_t = constp.tile([P, 1], MMDT, tag="ones")
            nc.gpsimd.memset(ones_t[:], 1.0)
            tri_t = constp.tile([P, P], MMDT, tag="tri01")

            # x^T stays resident: it is both the projection input and the
            # score matmul's K-side operand (A-fold).
            xt_t = projp.tile([P, KD * S], MMDT, tag="xt", name="xt")
            qt_sb = [projp.tile([P, S], MMDT, tag=f"qt{m}", name=f"qt{m}")
                     for m in range(KD)]
            v_sb = [projp.tile([P, D], MMDT, tag=f"v{i}", name=f"v{i}")
                    for i in range(NB)]

            def xs(c, k):  # x^T tile [128, 512]: d-block k, s-chunk c
                o = c * 2048 + k * CHUNK
                return xt_t[:, o:o + CHUNK]

            def xk(kb, kk):  # x^T tile [128, 128]: d-block kk, s-block kb
                o = (kb // 4) * 2048 + kk * CHUNK + (kb % 4) * P
                return xt_t[:, o:o + P]

            # ---- Phase A+B: loads + projections ----
            with tc.tile_pool(name="xw", bufs=1) as xwp:
                a_t = xwp.tile([P, KD * D], MMDT, tag="a", name="a")
                wv_t = xwp.tile([P, KD * D], MMDT, tag="wv", name="wv")
                # chunk-0 x and the A pack land first, spread across four
                # engines' DMA queues so the first projection starts ASAP
                for k in range(KD):
                    nc.sync.dma_start(
                        out=xt_t[:, k * CHUNK:(k + 1) * CHUNK],
                        in_=xp[:, k * CHUNK:(k + 1) * CHUNK])
                    nc.scalar.dma_start(
                        out=a_t[:, k * D:(k + 1) * D],
                        in_=ap[:, k * D:(k + 1) * D])
                nc.sync.dma_start(out=cst[:], in_=csts[:])
                nc.scalar.dma_start(out=wv_t[:], in_=wvp[:])
                nc.scalar.dma_start(out=tri_t[:], in_=tri01[:])
                for c in range(1, NCH):
                    for k in range(KD):
                        o = c * 2048 + k * CHUNK
                        nc.sync.dma_start(
                            out=xt_t[:, o:o + CHUNK], in_=xp[:, o:o + CHUNK])

                # Q~^T: [d_out block m, s chunk c] = sum_k A[k][:,m]^T x^T[k][:,c]
                for c in range(NCH):
                    for m in range(KD):
                        ps = stp.tile([P, CHUNK], F32, tag="pst")
                        for k in range(KD):
                            nc.tensor.matmul(
                                ps[:], a_t[:, k * D + m * P:k * D + (m + 1) * P],
                                xs(c, k), start=(k == 0), stop=(k == KD - 1))
                        nc.vector.tensor_copy(
                            qt_sb[m][:, c * CHUNK:(c + 1) * CHUNK], ps[:])
                    # V: [s block i, d] = sum_k x^T[k][:, i]^T wv[k]
                    for ii in range(4):
                        i = 4 * c + ii
                        ps = stp.tile([P, D], F32, tag="pst")
                        for k in range(KD):
                            nc.tensor.matmul(
                                ps[:], xt_t[:, c * 2048 + k * CHUNK + ii * P:
                                            c * 2048 + k * CHUNK + (ii + 1) * P],
                                wv_t[:, k * D:(k + 1) * D],
                                start=(k == 0), stop=(k == KD - 1))
                        nc.vector.tensor_copy(v_sb[i][:], ps[:])

            # ---- Phase C: attention per query chunk, 1-block ST lookahead ----
            def emit_norm_out(c, j, sums, sums_sb, ot):
                # sums j-slice -> per-partition [128,1] -> recip -> scale -> store
                nc.vector.tensor_copy(
                    sums_sb[0:1, j * P:(j + 1) * P],
                    sums[0:1, j * P:(j + 1) * P])
                eng = nc.sync if j % 2 == 0 else nc.scalar
                sums_tj = sumtp.tile([P, 1], F32, tag=f"sumt{j}",
                                     name=f"sumt{j}")
                eng.dma_start(out=sums_tj[:],
                              in_=sums_sb[0:1, j * P:(j + 1) * P])
                recip_j = sumtp.tile([P, 1], F32, tag=f"recip{j}",
                                     name=f"recip{j}")
                nc.vector.reciprocal(recip_j[:], sums_tj[:])
                if j % 2 == 0:
                    nc.vector.tensor_scalar_mul(ot[j][:], ot[j][:], recip_j[:])
                else:
                    nc.scalar.activation(
                        ot[j][:], ot[j][:],
                        mybir.ActivationFunctionType.Copy,
                        scale=recip_j[:])
                r0 = (4 * c + j) * P
                eng.dma_start(out=out[r0:r0 + P, :], in_=ot[j][:])

            for c in range(NCH):
                av = [avp.tile([P, D], F32, tag=f"av{j}", name=f"av{j}")
                      for j in range(4)]
                ot = [oevp.tile([P, D], F16, tag=f"ot{j}", name=f"ot{j}")
                      for j in range(4)]
                sums = sump.tile([1, CHUNK], F32, tag="sums")
                sums_sb = sumtp.tile([1, CHUNK], F32, tag="sums_sb")
                nkb = 4 * c + 4  # causal: sk blocks 0 .. 4c+3

                def emit_st(k):
                    # ST chunk [sk=128, sq<=512] = sum_d x^T[d,sk]^T Q~^T[d,sq];
                    # exp with pad bias; 0/1 tri multiply on the diagonal block
                    m = k - 4 * c
                    lo = max(m, 0) * P
                    st = stp.tile([P, CHUNK], F32, tag="pst")
                    for kk in range(KD):
                        nc.tensor.matmul(
                            st[:, lo:CHUNK], xk(k, kk),
                            qt_sb[kk][:, c * CHUNK + lo:(c + 1) * CHUNK],
                            start=(kk == 0), stop=(kk == KD - 1))
                    pt = ptp.tile([P, CHUNK], MMDT, tag="pt")
                    nc.scalar.activation(
                        pt[:, lo:CHUNK], st[:, lo:CHUNK],
                        mybir.ActivationFunctionType.Exp,
                        bias=cst[:, k:k + 1], scale=1.0)
                    if m >= 0:
                        nc.vector.tensor_mul(
                            pt[:, m * P:(m + 1) * P],
                            pt[:, m * P:(m + 1) * P], tri_t[:])
                    return pt

                pt_k = emit_st(0)
                for k in range(nkb):
                    m = k - 4 * c
                    lo = max(m, 0) * P
                    # lookahead: next block's ST goes to the PE queue before
                    # this block's sums/AV (which wait on the scalar engine's
                    # exp) — keeps the in-order PE busy during the exp.
                    pt_next = emit_st(k + 1) if k + 1 < nkb else None
                    nc.tensor.matmul(
                        sums[0:1, lo:CHUNK], ones_t[:], pt_k[:, lo:CHUNK],
                        start=(k == 0), stop=(k == nkb - 1))
                    for j in range(4):
                        if k <= 4 * c + j:
                            nc.tensor.matmul(
                                av[j][:], pt_k[:, j * P:(j + 1) * P], v_sb[k][:],
                                start=(k == 0), stop=(k == 4 * c + j))
                    if m >= 0:
                        # av[m] complete: evacuate unnormalized now to free
                        # its PSUM bank for the next chunk
                        if m % 2 == 0:
                            nc.scalar.activation(
                                ot[m][:], av[m][:],
                                mybir.ActivationFunctionType.Copy)
                        else:
                            nc.vector.tensor_copy(ot[m][:], av[m][:])
                        # sums cols [m*128:(m+1)*128] got their last
                        # contribution at this k (later blocks trim past
                        # them): normalize + store this q-subblock now so
                        # the end-of-chunk tail only holds the last one
                        emit_norm_out(c, m, sums, sums_sb, ot)
                    pt_k = pt_next
    nc.compile()
    return nc


def _get_nc():
    if "nc" not in _cache:
        _cache["nc"] = _build()
    return _cache["nc"]


def _in_maps(batch, wq, bq, wk, bk, wv, bv, lengths):
    wq64 = wq.astype(np.float64)
    wk64 = wk.astype(np.float64)
    a_eff = (wq64.T @ wk64) / np.sqrt(D)        # [D, D]
    colvec = (wk64.T @ bq.astype(np.float64)) / np.sqrt(D)

    def packw(m):
        # [p, k*512 + j] = m[k*128 + p, j]
        return np.ascontiguousarray(
            m.reshape(KD, P, D).transpose(1, 0, 2).reshape(P, KD * D)
        ).astype(np.float16)

    app = packw(a_eff)
    wvpp = packw(wv.astype(np.float64).T)
    tri01 = np.ascontiguousarray(np.where(
        np.arange(P)[:, None] <= np.arange(P)[None, :],
        np.float16(1), np.float16(0)))
    cols = np.arange(S)
    maps = []
    for b in range(N_CORES):
        # xp[p, c*2048 + k*512 + j] = x[c*512 + j, k*128 + p]
        xb = batch[b].astype(np.float16)
        xpk = np.ascontiguousarray(
            xb.reshape(NCH, CHUNK, KD, P).transpose(3, 0, 2, 1).reshape(P, KD * S))
        colbias = batch[b].astype(np.float64) @ colvec
        padb = np.where(cols < int(lengths[b]), 0.0, NEG) + colbias
        cst_b = np.ascontiguousarray(padb.reshape(NB, P).T.astype(np.float32))
        maps.append({"xp": xpk, "ap": app, "wvp": wvpp,
                     "csts": cst_b, "tri01": tri01})
    return maps


def _execute(in_maps, trace=False):
    nc = _get_nc()
    # always install: run_bass_kernel_spmd also honours a BASS_TRACE env var,
    # and would crash importing antenv.axon_hooks if unregistered
    _install_ntff_hook()
    return run_bass_kernel_spmd(nc, in_maps, list(range(N_CORES)), trace=trace)


def _install_ntff_hook():
    """The agent image's antenv lacks axon_hooks; register the NTFF profile
    hook ourselves so trace=True yields exec_time_ns."""
    import sys, types
    if "antenv.axon_hooks" in sys.modules:
        return
    try:
        import trn_agent_boot.trn_boot as tb
        hook = tb._ntff_profile_via_ctypes("/opt/axon/libaxon_pjrt.so")
    except Exception:
        return
    mod = types.ModuleType("antenv.axon_hooks")
    mod._hook = hook
    mod.get_axon_ntff_profile_hook = lambda: mod._hook
    mod.set_axon_ntff_profile_hook = lambda h: setattr(mod, "_hook", h)
    sys.modules["antenv.axon_hooks"] = mod
    try:
        import antenv
        antenv.axon_hooks = mod
    except Exception:
        pass


def kernel(batch, wq, bq, wk, bk, wv, bv, lengths):
    batch = np.asarray(batch)
    wq, bq = np.asarray(wq), np.asarray(bq)
    wk, bk = np.asarray(wk), np.asarray(bk)
    wv, bv = np.asarray(wv), np.asarray(bv)
    lengths = np.asarray(lengths)
    maps = _in_maps(batch, wq, bq, wk, bk, wv, bv, lengths)
    res = _execute(maps, trace=False)
    outs = [np.asarray(res.results[b]["out"]) for b in range(N_CORES)]
    full = np.stack(outs, axis=0).astype(np.float32)
    full += bv.astype(np.float32)[None, None, :]
    return full
